# revision 10
# baseline (speedup 1.0000x reference)
"""Trainium2 Bass kernel for nn_EntityMapping (P=16 independent MLPs over a
shared entity batch).

Sharding: the 16 partition-MLPs are split across 8 NeuronCores (2 per core,
expert-parallel); the embedding batch is replicated. Activations are kept
feature-major [feature, batch] on-chip so every layer is a chain of
128x128-stationary matmuls with the batch streaming through the PE array.

All matmuls run in fp8 (e4m3) DoubleRow perf mode: one instruction contracts
over 256 rows (two 128-k-chunks) and streams 512 batch columns at 1 col/PE
cycle — measured 216ns issue-to-issue at 2.4GHz, i.e. ~99% of the 157TF/s
fp8 peak (2x fp32r per unit time). Inputs/weights are quantized host-side
with power-of-2 scales chosen so each layer's PSUM lands directly in the
next layer's fp8 units (SE*SW0 = SH1, SH1*SW1 = SH2): the relu is then a
pure (x + bias, max 0) with no rescale — a single activation on ACT or a
single dual-op tensor_scalar on DVE. GPSIMD cannot read PSUM, so relu work
is split ACT/DVE (9:7, matching their measured per-op cost); to amortize
the per-instruction PSUM-access overhead, batch chunks are processed in
PAIRS: the same j-tile of two adjacent 512-column chunks lands in adjacent
PSUM banks and one flat [128, 1024] relu (the bias is per-feature, so it is
shared) covers both. The final w2-dot runs as DoubleRow matmuls against a
64-column zero-padded stationary (dual-fp8 Ldweights requires >=32 columns
and dst partition 0) with MLP p's vector at column p, all four accumulating
into one PSUM region, so the two MLPs' dots land on adjacent partitions and
a single sigmoid [2, 1024] finishes both MLPs and both chunks.

Measured: ~147.5-148.5us HW exec (vs 269.6us fp32r baseline, 1.83x), built
from ~124us PE streaming floor (576 matmuls x 216ns) + ~9us Tile init +
~3.5us first-DMA bridge (covered by warmup junk matmuls) + ~3us HAM
half-clock ramp + ~5us tail (final sigmoid chain + Tile drain). All eight
PSUM banks rotate through one shared 4-tile pool; eT is fully SBUF-resident
(32KB/partition in fp8) via the ACT HW-DGE queue, weights via the SP queue.
Quantization error (validated on host vs the fp32 reference): rel_fro
~3.2e-3 (gate 2e-2), max |pre-fp8 activation| ~104 of the 240 e4m3 range.
"""

import os
import sys

import numpy as np

if "jax" not in sys.modules and os.environ.get("JAX_PLATFORMS") == "cpu":
    # don't let a cpu pin hide the axon/neuron backend the kernel runs on
    os.environ["JAX_PLATFORMS"] = ""

try:
    import concourse.bass as bass  # noqa: F401
except ImportError:  # harness runs kernel.py from a bare directory
    sys.path.insert(0, "/opt/trn_rl_repo")

import ml_dtypes
import concourse.mybir as mybir
import concourse.tile as tile
from concourse import bacc
from concourse.bass_utils import run_bass_kernel_spmd

F32 = mybir.dt.float32
F8 = mybir.dt.float8e4
RELU = mybir.ActivationFunctionType.Relu
SIGMOID = mybir.ActivationFunctionType.Sigmoid
DR = mybir.MatmulPerfMode.DoubleRow
ADD = mybir.AluOpType.add
MAX = mybir.AluOpType.max

F8NP = ml_dtypes.float8_e4m3  # TRN-style e4m3 (max +-240), not OCP e4m3fn

P_TOTAL = 16  # independent MLP partitions
E = 512  # entity/embedding dim
H = 512  # hidden dim
N = 8192  # batch (entities)
N_CORES = 8
P_PER = P_TOTAL // N_CORES  # 2 MLPs per core
JC = 4  # 128-wide output-feature tiles per layer
KP = 2  # DoubleRow pairs per 512-deep contraction (2 x 256)
NCH = 512  # batch columns per chunk (= PSUM bank of fp32)
NDC = N // (2 * NCH)  # 8 double-chunks
NW = P_PER * JC * KP  # 16 DoubleRow weight tiles [128,2,128] per layer

# power-of-2 quantization scales; SE*SW0 = SH1 and SH1*SW1 = SH2 make the
# PSUM arrive already in the next layer's fp8 units (relu needs no rescale)
SE, SW0, SW1, SW2 = 2.0, 16.0, 2.0, 128.0
SH1 = SE * SW0  # 32
SH2 = SH1 * SW1  # 64
SC2 = 1.0 / (SH2 * SW2)  # 1/8192, applied inside the sigmoid activation

WARMUP_MM = 8  # junk 512-col matmuls bridging the first-DMA window (HAM ramp)


def _build():
    nc = bacc.Bacc(
        "TRN2", target_bir_lowering=False, debug=False, num_devices=N_CORES
    )
    # All inputs pre-quantized and pre-packed on host into SBUF layout:
    eT_dram = nc.dram_tensor("eT", [128, KP * 2 * N], F8, kind="ExternalInput")
    w0_dram = nc.dram_tensor("w0", [128, NW * 2 * 128], F8, kind="ExternalInput")
    w1_dram = nc.dram_tensor("w1", [128, NW * 2 * 128], F8, kind="ExternalInput")
    # dual-fp8 Ldweights needs a stationary free dim >= 32 and DoubleRow
    # matmuls need dst partition 0 (s3d3_mm_valid_dst_partition): w2 is
    # padded to 64 zero columns with MLP p's vector at column p, so one
    # accumulated PSUM region holds the two dots on adjacent partitions 0/1
    # and one sigmoid covers both
    w2_dram = nc.dram_tensor(
        "w2", [128, P_PER * KP * 2 * 64], F8, kind="ExternalInput"
    )
    # all biases ride in ONE tensor (one DGE descriptor push instead of
    # three): cols 0:8 = b0*SH1, 8:16 = b1*SH2, col 16 = b2 on partitions
    # 0/1 — pushed right after w0 piece 0 so the first relus never wait
    bb_dram = nc.dram_tensor("bb", [128, 2 * P_PER * JC + 1], F32,
                             kind="ExternalInput")
    out_dram = nc.dram_tensor("out", [P_PER, N], F32, kind="ExternalOutput")

    eT_v = eT_dram.rearrange("p (kp two n) -> p kp two n", kp=KP, two=2)

    with tile.TileContext(nc) as tc:
        with (
            tc.tile_pool(name="wconst", bufs=1) as wconst,
            tc.tile_pool(name="warm", bufs=1) as warm_pool,
            tc.tile_pool(name="et", bufs=1) as et_pool,
            tc.tile_pool(name="h1", bufs=4) as h1_pool,
            tc.tile_pool(name="h2", bufs=4) as h2_pool,
            tc.tile_pool(name="osb", bufs=3) as out_pool,
            # one shared 4-tile PSUM rotation (2 banks each = all 8 banks):
            # L2 uses the same pool as the layer matmuls, so buffering goes
            # where the pipeline needs it instead of a dedicated L2 bank
            tc.tile_pool(name="mmps", bufs=4, space="PSUM") as ps_mm,
        ):
            # --- PE warmup: junk DoubleRow matmuls on a memset tile so the
            # PE clock is ramped when the first real matmul issues (the
            # memset is kept small so the first junk matmul can issue as
            # early as possible after the framework preamble) ---
            wm = warm_pool.tile([128, 2, 640], F8, tag="wm")
            nc.gpsimd.memset(wm[:], 0.0)
            ps_warm = ps_mm.tile([128, 2, NCH], F32, tag="mm")
            for i in range(WARMUP_MM):
                nc.tensor.matmul(
                    ps_warm[:, 0, :],
                    wm[:, :, 0:128],
                    wm[:, :, 128:640],
                    start=(i == 0),
                    stop=(i == WARMUP_MM - 1),
                    perf_mode=DR,
                )

            # --- whole eT resident in SBUF (32KB/partition in fp8); issued
            # on the ACT HW-DGE queue (ACT is idle this early), column-split
            # so chunk 0 unblocks immediately ---
            # chunk-0 columns split across the two HW-DGE queues (SP carries
            # q0 between the weight pieces, ACT carries q1) so the first
            # real matmul unblocks ~3us earlier; the eT bulk then rides the
            # ACT queue, serialized behind block 0 so it can't starve it
            et = et_pool.tile([128, KP, 2, N], F8, tag="et")

            # --- weights + consts on the SP HW-DGE queue, in consumption
            # order: w0 piece 0, biases, w2, w0 piece 1, w1 pieces ---
            w0s = wconst.tile([128, NW, 2, 128], F8, tag="w0")
            w1s = wconst.tile([128, NW, 2, 128], F8, tag="w1")
            w2s = wconst.tile([128, P_PER, KP, 2, 64], F8, tag="w2")
            bbs = wconst.tile([128, 2 * P_PER * JC + 1], F32, tag="bb")

            w0sv = w0s[:].rearrange("p i two m -> p (i two m)")
            w1sv = w1s[:].rearrange("p i two m -> p (i two m)")
            PIECE = JC * KP * 2 * 128  # one per-MLP half-layer = 2KB fp8

            def w_dma(dst_flat, src, p):
                q0 = p * PIECE
                nc.sync.dma_start(
                    dst_flat[:, q0 : q0 + PIECE],
                    src[:, q0 : q0 + PIECE],
                )

            # dc0's eT as four contiguous per-(q,t) pieces (strided 512B
            # runs measured 2-3x slower and starved the early stream), then
            # the bulk as two big strided descriptors whose 3.5/7KB runs go
            # full rate — only 6 pushes total so the ACT engine is free for
            # relu work by ~10.5us
            for q in range(KP):
                for t in range(2):
                    nc.scalar.dma_start(
                        et[:, q, t, 0:1024], eT_v[:, q, t, 0:1024]
                    )
            for lo, hi in ((1024, 4096), (4096, N)):
                nc.scalar.dma_start(et[:, :, :, lo:hi], eT_v[:, :, :, lo:hi])
            # SP queue, need-ordered: w0p0 (first matmul), biases (first
            # relu), w2 (first l2), w0p1, w1 pieces
            w_dma(w0sv, w0_dram, 0)
            nc.sync.dma_start(bbs[:], bb_dram[:])
            nc.sync.dma_start(
                w2s[:].rearrange("p a q two m -> p (a q two m)"), w2_dram[:]
            )
            w_dma(w0sv, w0_dram, 1)
            w_dma(w1sv, w1_dram, 0)
            w_dma(w1sv, w1_dram, 1)

            # --- relu dispatch: 9 ACT / 7 DVE per double-chunk (measured
            # 1105ns vs 1541ns per op; ACT also owns the sigmoid) ---
            relu_seq = "DADADADADADADADA"
            relu_ctr = [0]

            def relu(dst, ps, bias):
                # flat [128, 1024] APs (both sides are contiguous)
                dst = dst.rearrange("p a b -> p (a b)")
                src = ps[:].rearrange("p a b -> p (a b)")
                eng = relu_seq[relu_ctr[0] % 16]
                relu_ctr[0] += 1
                if eng == "D":
                    nc.vector.tensor_scalar(dst, src, bias, 0.0, ADD, MAX)
                else:
                    nc.scalar.activation(dst, src, RELU, bias=bias)

            # h tiles hold one double-chunk: [feature128, j, cc, col]
            def l0(p, dc):
                n0 = dc * 2 * NCH
                h1 = h1_pool.tile([128, JC, 2, NCH], F8, tag="h1")
                for j in range(JC):
                    ps = ps_mm.tile([128, 2, NCH], F32, tag="mm")
                    for cc in range(2):
                        nn = n0 + cc * NCH
                        for q in range(KP):
                            wi = (p * JC + j) * KP + q
                            nc.tensor.matmul(
                                ps[:, cc, :],
                                w0s[:, wi, :, :],
                                et[:, q, :, nn : nn + NCH],
                                start=(q == 0),
                                stop=(q == KP - 1),
                                perf_mode=DR,
                            )
                    col = p * JC + j
                    relu(h1[:, j, :, :], ps, bbs[:, col : col + 1])
                return h1

            def l1(p, h1):
                h2 = h2_pool.tile([128, JC, 2, NCH], F8, tag="h2")
                for j in range(JC):
                    ps = ps_mm.tile([128, 2, NCH], F32, tag="mm")
                    for cc in range(2):
                        for q in range(KP):
                            wi = (p * JC + j) * KP + q
                            nc.tensor.matmul(
                                ps[:, cc, :],
                                w1s[:, wi, :, :],
                                h1[:, 2 * q : 2 * q + 2, cc, :],
                                start=(q == 0),
                                stop=(q == KP - 1),
                                perf_mode=DR,
                            )
                    col = 8 + p * JC + j
                    relu(h2[:, j, :, :], ps, bbs[:, col : col + 1])
                return h2

            def l2(dc, h2_by_p):
                n0 = dc * 2 * NCH
                r = ps_mm.tile([128, 2, NCH], F32, tag="mm")
                o = out_pool.tile([2, 2, NCH], F32, tag="o")
                b2ap = bbs[0:2, 2 * P_PER * JC : 2 * P_PER * JC + 1]
                last = dc == NDC - 1
                for cc in range(2):
                    for p, h2 in enumerate(h2_by_p):
                        for q in range(KP):
                            nc.tensor.matmul(
                                r[0:64, cc, :],
                                w2s[:, p, q, :, :],
                                h2[:, 2 * q : 2 * q + 2, cc, :],
                                start=(p == 0 and q == 0),
                                stop=(p == P_PER - 1 and q == KP - 1),
                                perf_mode=DR,
                            )
                    if last:
                        # per-chunk sigmoid emitted right after this chunk's
                        # matmuls: the cc0 sigmoid+DMA overlap cc1's matmuls
                        # (PSUM dep tracking is bank-aware), so the serial
                        # tail is one [2,512] sigmoid + one 2-row DMA
                        nc.scalar.activation(
                            o[:, cc, :], r[0:2, cc, :],
                            SIGMOID, bias=b2ap, scale=SC2,
                        )
                        nc.sync.dma_start(
                            out_dram[0:2, n0 + cc * NCH : n0 + (cc + 1) * NCH],
                            o[:, cc, :],
                        )
                if not last:
                    nc.scalar.activation(
                        o[:].rearrange("p a b -> p (a b)"),
                        r[0:2, :, :].rearrange("p a b -> p (a b)"),
                        SIGMOID, bias=b2ap, scale=SC2,
                    )
                    nc.sync.dma_start(
                        out_dram[0:2, n0 : n0 + 2 * NCH],
                        o[:].rearrange("p a b -> p (a b)"),
                    )

            # --- software-pipelined main loop over double-chunks: dc+1's L0
            # runs between dc's L1 and L2, giving every relu a full PE-block
            # of slack before its consumer ---
            h1s = [l0(0, 0), l0(1, 0)]
            for dc in range(NDC):
                h2s = [l1(0, h1s[0]), l1(1, h1s[1])]
                if dc + 1 < NDC:
                    h1s = [l0(0, dc + 1), l0(1, dc + 1)]
                l2(dc, h2s)

    nc.compile()
    return nc


_NC_CACHE = None


def _get_nc():
    global _NC_CACHE
    if _NC_CACHE is None:
        _NC_CACHE = _build()
    return _NC_CACHE


def _q8(x, scale):
    return (np.asarray(x, dtype=np.float32) * scale).astype(F8NP)


def _make_in_maps(e_embedding, W0, b0, W1, b1, W2, b2):
    e = np.asarray(e_embedding, dtype=np.float32)
    W0 = np.asarray(W0, dtype=np.float32)
    b0 = np.asarray(b0, dtype=np.float32)
    W1 = np.asarray(W1, dtype=np.float32)
    b1 = np.asarray(b1, dtype=np.float32)
    W2 = np.asarray(W2, dtype=np.float32)
    b2 = np.asarray(b2, dtype=np.float32)

    # eT [E, N] -> [ki, kp, two, n] fp8 (replicated to all cores)
    eTq = np.ascontiguousarray(
        _q8(e.T, SE).reshape(KP, 2, 128, N).transpose(2, 0, 1, 3).reshape(128, -1)
    )

    def wpack(W, s):  # [pp, 512, 512] -> [ki, (p j q two m)] fp8
        return np.ascontiguousarray(
            _q8(W, s)
            .reshape(P_PER, KP, 2, 128, JC, 128)
            .transpose(3, 0, 4, 1, 2, 5)
            .reshape(128, -1)
        )

    in_maps = []
    for cid in range(N_CORES):
        sl = slice(P_PER * cid, P_PER * (cid + 1))
        w0t = wpack(W0[sl], SW0)
        w1t = wpack(W1[sl], SW1)
        w2q = (
            _q8(W2[sl, :, 0], SW2)
            .reshape(P_PER, KP, 2, 128)
            .transpose(3, 0, 1, 2)
        )  # [128, p, q, two]
        w2t = np.zeros((128, P_PER, KP, 2, 64), dtype=F8NP)
        for p in range(P_PER):
            w2t[:, p, :, :, p] = w2q[:, p]
        w2t = np.ascontiguousarray(w2t.reshape(128, -1))
        bbt = np.zeros((128, 2 * P_PER * JC + 1), dtype=np.float32)
        bbt[:, 0:8] = (
            (b0[sl] * SH1).reshape(P_PER, JC, 128).transpose(2, 0, 1).reshape(128, -1)
        )
        bbt[:, 8:16] = (
            (b1[sl] * SH2).reshape(P_PER, JC, 128).transpose(2, 0, 1).reshape(128, -1)
        )
        bbt[0:2, 16] = b2[sl, 0]
        in_maps.append(
            {"eT": eTq, "w0": w0t, "w1": w1t, "w2": w2t,
             "bb": np.ascontiguousarray(bbt)}
        )
    return in_maps


def kernel_with_results(trace=False, **inputs):
    nc = _get_nc()
    in_maps = _make_in_maps(**inputs)
    try:
        res = run_bass_kernel_spmd(
            nc, in_maps, core_ids=list(range(N_CORES)), trace=trace
        )
    except Exception:
        # the first PJRT compile in a fresh container can fail transiently;
        # one retry reuses the primed NEFF cache
        res = run_bass_kernel_spmd(
            nc, in_maps, core_ids=list(range(N_CORES)), trace=trace
        )
    full = np.concatenate([r["out"] for r in res.results], axis=0)  # [16, N]
    out = np.ascontiguousarray(full.T).astype(np.float32)  # [N, 16]
    return out, res


def kernel(**inputs):
    out, _ = kernel_with_results(trace=False, **inputs)
    return out



# revision 13
# speedup vs baseline: 1.0283x; 1.0283x over previous
"""Trainium2 Bass kernel for nn_EntityMapping (P=16 independent MLPs over a
shared entity batch).

Sharding: the 16 partition-MLPs are split across 8 NeuronCores (2 per core,
expert-parallel); the embedding batch is replicated. Activations are kept
feature-major [feature, batch] on-chip so every layer is a chain of
128x128-stationary matmuls with the batch streaming through the PE array.

All matmuls run in fp8 (e4m3) DoubleRow perf mode: one instruction contracts
over 256 rows (two 128-k-chunks) and streams 512 batch columns at 1 col/PE
cycle — measured 216ns issue-to-issue at 2.4GHz, i.e. ~99% of the 157TF/s
fp8 peak (2x fp32r per unit time). Inputs/weights are quantized host-side
with power-of-2 scales chosen so each layer's PSUM lands directly in the
next layer's fp8 units (SE*SW0 = SH1, SH1*SW1 = SH2): the relu is then a
pure (x + bias, max 0) with no rescale — a single activation on ACT or a
single dual-op tensor_scalar on DVE. GPSIMD cannot read PSUM, so relu work
is split ACT/DVE (9:7, matching their measured per-op cost); to amortize
the per-instruction PSUM-access overhead, batch chunks are processed in
PAIRS: the same j-tile of two adjacent 512-column chunks lands in adjacent
PSUM banks and one flat [128, 1024] relu (the bias is per-feature, so it is
shared) covers both. The final w2-dot runs as DoubleRow matmuls against a
64-column zero-padded stationary (dual-fp8 Ldweights requires >=32 columns
and dst partition 0) with MLP p's vector at column p, all four accumulating
into one PSUM region, so the two MLPs' dots land on adjacent partitions and
a single sigmoid [2, 1024] finishes both MLPs and both chunks.

Measured: ~147.5-148.5us HW exec (vs 269.6us fp32r baseline, 1.83x), built
from ~124us PE streaming floor (576 matmuls x 216ns) + ~9us Tile init +
~3.5us first-DMA bridge (covered by warmup junk matmuls) + ~3us HAM
half-clock ramp + ~5us tail (final sigmoid chain + Tile drain). All eight
PSUM banks rotate through one shared 4-tile pool; eT is fully SBUF-resident
(32KB/partition in fp8) via the ACT HW-DGE queue, weights via the SP queue.
Quantization error (validated on host vs the fp32 reference): rel_fro
~3.2e-3 (gate 2e-2), max |pre-fp8 activation| ~104 of the 240 e4m3 range.
"""

import os
import sys

import numpy as np

if "jax" not in sys.modules and os.environ.get("JAX_PLATFORMS") == "cpu":
    # don't let a cpu pin hide the axon/neuron backend the kernel runs on
    os.environ["JAX_PLATFORMS"] = ""

try:
    import concourse.bass as bass  # noqa: F401
except ImportError:  # harness runs kernel.py from a bare directory
    sys.path.insert(0, "/opt/trn_rl_repo")

import ml_dtypes
import concourse.mybir as mybir
import concourse.tile as tile
from concourse import bacc
from concourse.bass_utils import run_bass_kernel_spmd

F32 = mybir.dt.float32
F8 = mybir.dt.float8e4
RELU = mybir.ActivationFunctionType.Relu
SIGMOID = mybir.ActivationFunctionType.Sigmoid
DR = mybir.MatmulPerfMode.DoubleRow
ADD = mybir.AluOpType.add
MAX = mybir.AluOpType.max

F8NP = ml_dtypes.float8_e4m3  # TRN-style e4m3 (max +-240), not OCP e4m3fn

P_TOTAL = 16  # independent MLP partitions
E = 512  # entity/embedding dim
H = 512  # hidden dim
N = 8192  # batch (entities)
N_CORES = 8
P_PER = P_TOTAL // N_CORES  # 2 MLPs per core
JC = 4  # 128-wide output-feature tiles per layer
KP = 2  # DoubleRow pairs per 512-deep contraction (2 x 256)
NCH = 512  # batch columns per chunk (= PSUM bank of fp32)
NDC = N // (2 * NCH)  # 8 double-chunks
NW = P_PER * JC * KP  # 16 DoubleRow weight tiles [128,2,128] per layer

# power-of-2 quantization scales; SE*SW0 = SH1 and SH1*SW1 = SH2 make the
# PSUM arrive already in the next layer's fp8 units (relu needs no rescale)
SE, SW0, SW1, SW2 = 2.0, 16.0, 2.0, 128.0
SH1 = SE * SW0  # 32
SH2 = SH1 * SW1  # 64
SC2 = 1.0 / (SH2 * SW2)  # 1/8192, applied inside the sigmoid activation

WARMUP_MM = 10  # junk 512-col matmuls bridging the first-DMA window (HAM ramp)


def _build():
    nc = bacc.Bacc(
        "TRN2", target_bir_lowering=False, debug=False, num_devices=N_CORES
    )
    # All inputs pre-quantized and pre-packed on host into SBUF layout:
    eT_dram = nc.dram_tensor("eT", [128, KP * 2 * N], F8, kind="ExternalInput")
    w0_dram = nc.dram_tensor("w0", [128, NW * 2 * 128], F8, kind="ExternalInput")
    w1_dram = nc.dram_tensor("w1", [128, NW * 2 * 128], F8, kind="ExternalInput")
    # dual-fp8 Ldweights needs a stationary free dim >= 32 and DoubleRow
    # matmuls need dst partition 0 (s3d3_mm_valid_dst_partition): w2 is
    # padded to 64 zero columns with MLP p's vector at column p, so one
    # accumulated PSUM region holds the two dots on adjacent partitions 0/1
    # and one sigmoid covers both
    w2_dram = nc.dram_tensor(
        "w2", [128, P_PER * KP * 2 * 64], F8, kind="ExternalInput"
    )
    # all biases ride in ONE tensor (one DGE descriptor push instead of
    # three): cols 0:8 = b0*SH1, 8:16 = b1*SH2, col 16 = b2 on partitions
    # 0/1 — pushed right after w0 piece 0 so the first relus never wait
    bb_dram = nc.dram_tensor("bb", [128, 2 * P_PER * JC + 1], F32,
                             kind="ExternalInput")
    out_dram = nc.dram_tensor("out", [P_PER, N], F32, kind="ExternalOutput")

    eT_v = eT_dram.rearrange("p (kp two n) -> p kp two n", kp=KP, two=2)

    with tile.TileContext(nc) as tc:
        with (
            tc.tile_pool(name="wconst", bufs=1) as wconst,
            tc.tile_pool(name="warm", bufs=1) as warm_pool,
            tc.tile_pool(name="et", bufs=1) as et_pool,
            tc.tile_pool(name="h1", bufs=4) as h1_pool,
            tc.tile_pool(name="h2", bufs=4) as h2_pool,
            tc.tile_pool(name="osb", bufs=3) as out_pool,
            # one shared 4-tile PSUM rotation (2 banks each = all 8 banks):
            # L2 uses the same pool as the layer matmuls, so buffering goes
            # where the pipeline needs it instead of a dedicated L2 bank
            tc.tile_pool(name="mmps", bufs=4, space="PSUM") as ps_mm,
        ):
            # --- PE warmup: junk DoubleRow matmuls on a memset tile so the
            # PE clock is ramped when the first real matmul issues (the
            # memset is kept small so the first junk matmul can issue as
            # early as possible after the framework preamble) ---
            # minimal 8B memset just to allocate the tile: the junk
            # matmuls otherwise read uninitialized SBUF — TRN-style e4m3
            # has no NaN/Inf encodings so garbage stays finite, and the
            # junk PSUM is never read; skipping the full ~1.2us GPSIMD
            # memset lets the HAM ramp start that much earlier
            wm = warm_pool.tile([128, 2, 640], F8, tag="wm")
            nc.gpsimd.memset(wm[:, 0, 0:8], 0.0)
            ps_warm = ps_mm.tile([128, 2, NCH], F32, tag="mm")
            for i in range(WARMUP_MM):
                nc.tensor.matmul(
                    ps_warm[:, 0, :],
                    wm[:, :, 0:128],
                    wm[:, :, 128:640],
                    start=(i == 0),
                    stop=(i == WARMUP_MM - 1),
                    perf_mode=DR,
                )

            # --- whole eT resident in SBUF (32KB/partition in fp8); issued
            # on the ACT HW-DGE queue (ACT is idle this early), column-split
            # so chunk 0 unblocks immediately ---
            # chunk-0 columns split across the two HW-DGE queues (SP carries
            # q0 between the weight pieces, ACT carries q1) so the first
            # real matmul unblocks ~3us earlier; the eT bulk then rides the
            # ACT queue, serialized behind block 0 so it can't starve it
            et = et_pool.tile([128, KP, 2, N], F8, tag="et")

            # --- weights + consts on the SP HW-DGE queue, in consumption
            # order: w0 piece 0, biases, w2, w0 piece 1, w1 pieces ---
            w0s = wconst.tile([128, NW, 2, 128], F8, tag="w0")
            w1s = wconst.tile([128, NW, 2, 128], F8, tag="w1")
            w2s = wconst.tile([128, P_PER, KP, 2, 64], F8, tag="w2")
            bbs = wconst.tile([128, 2 * P_PER * JC + 1], F32, tag="bb")

            w0sv = w0s[:].rearrange("p i two m -> p (i two m)")
            w1sv = w1s[:].rearrange("p i two m -> p (i two m)")
            PIECE = JC * KP * 2 * 128  # one per-MLP half-layer = 2KB fp8

            def w_dma(dst_flat, src, p):
                q0 = p * PIECE
                nc.sync.dma_start(
                    dst_flat[:, q0 : q0 + PIECE],
                    src[:, q0 : q0 + PIECE],
                )

            # dc0's eT as four contiguous per-(q,t) pieces (strided 512B
            # runs measured 2-3x slower and starved the early stream), then
            # the bulk as two big strided descriptors whose 3.5/7KB runs go
            # full rate — only 6 pushes total so the ACT engine is free for
            # relu work by ~10.5us
            for q in range(KP):
                for t in range(2):
                    nc.scalar.dma_start(
                        et[:, q, t, 0:1024], eT_v[:, q, t, 0:1024]
                    )
            for lo, hi in ((1024, 4096), (4096, N)):
                nc.scalar.dma_start(et[:, :, :, lo:hi], eT_v[:, :, :, lo:hi])
            # SP queue, need-ordered: both w0 pieces back-to-back (l0 p1
            # needs piece 1 by ~12us; anything between the pieces starves
            # it), then biases (first relu ~12.3us), w2, w1 pieces
            w_dma(w0sv, w0_dram, 0)
            w_dma(w0sv, w0_dram, 1)
            nc.sync.dma_start(bbs[:], bb_dram[:])
            nc.sync.dma_start(
                w2s[:].rearrange("p a q two m -> p (a q two m)"), w2_dram[:]
            )
            w_dma(w1sv, w1_dram, 0)
            w_dma(w1sv, w1_dram, 1)

            # --- relu dispatch: 9 ACT / 7 DVE per double-chunk (measured
            # 1105ns vs 1541ns per op; ACT also owns the sigmoid) ---
            relu_seq = "DADADADADADADADA"
            relu_ctr = [0]

            def relu(dst, ps, bias):
                # flat [128, 1024] APs (both sides are contiguous)
                dst = dst.rearrange("p a b -> p (a b)")
                src = ps[:].rearrange("p a b -> p (a b)")
                eng = relu_seq[relu_ctr[0] % 16]
                relu_ctr[0] += 1
                if eng == "D":
                    nc.vector.tensor_scalar(dst, src, bias, 0.0, ADD, MAX)
                else:
                    nc.scalar.activation(dst, src, RELU, bias=bias)

            # h tiles hold one double-chunk: [feature128, j, cc, col]
            def l0(p, dc):
                n0 = dc * 2 * NCH
                h1 = h1_pool.tile([128, JC, 2, NCH], F8, tag="h1")
                for j in range(JC):
                    ps = ps_mm.tile([128, 2, NCH], F32, tag="mm")
                    for cc in range(2):
                        nn = n0 + cc * NCH
                        for q in range(KP):
                            wi = (p * JC + j) * KP + q
                            nc.tensor.matmul(
                                ps[:, cc, :],
                                w0s[:, wi, :, :],
                                et[:, q, :, nn : nn + NCH],
                                start=(q == 0),
                                stop=(q == KP - 1),
                                perf_mode=DR,
                            )
                    col = p * JC + j
                    relu(h1[:, j, :, :], ps, bbs[:, col : col + 1])
                return h1

            def l1(p, h1):
                h2 = h2_pool.tile([128, JC, 2, NCH], F8, tag="h2")
                for j in range(JC):
                    ps = ps_mm.tile([128, 2, NCH], F32, tag="mm")
                    for cc in range(2):
                        for q in range(KP):
                            wi = (p * JC + j) * KP + q
                            nc.tensor.matmul(
                                ps[:, cc, :],
                                w1s[:, wi, :, :],
                                h1[:, 2 * q : 2 * q + 2, cc, :],
                                start=(q == 0),
                                stop=(q == KP - 1),
                                perf_mode=DR,
                            )
                    col = 8 + p * JC + j
                    relu(h2[:, j, :, :], ps, bbs[:, col : col + 1])
                return h2

            def l2(dc, h2_by_p):
                n0 = dc * 2 * NCH
                r = ps_mm.tile([128, 2, NCH], F32, tag="mm")
                o = out_pool.tile([2, 2, NCH], F32, tag="o")
                b2ap = bbs[0:2, 2 * P_PER * JC : 2 * P_PER * JC + 1]
                last = dc == NDC - 1
                # q-outer order: the q0 matmuls need only j0/j1 relus, so
                # the PE has ~0.9us of work in flight before the last l1
                # relus (j2/j3) must land — kills the dc7 wind-down stall
                for cc in range(2):
                    for q in range(KP):
                        for p, h2 in enumerate(h2_by_p):
                            nc.tensor.matmul(
                                r[0:64, cc, :],
                                w2s[:, p, q, :, :],
                                h2[:, 2 * q : 2 * q + 2, cc, :],
                                start=(p == 0 and q == 0),
                                stop=(p == P_PER - 1 and q == KP - 1),
                                perf_mode=DR,
                            )
                    if last:
                        # per-chunk sigmoid emitted right after this chunk's
                        # matmuls: the cc0 sigmoid+DMA overlap cc1's matmuls
                        # (PSUM dep tracking is bank-aware), so the serial
                        # tail is one [2,512] sigmoid + one 2-row DMA
                        nc.scalar.activation(
                            o[:, cc, :], r[0:2, cc, :],
                            SIGMOID, bias=b2ap, scale=SC2,
                        )
                        nc.sync.dma_start(
                            out_dram[0:2, n0 + cc * NCH : n0 + (cc + 1) * NCH],
                            o[:, cc, :],
                        )
                if not last:
                    nc.scalar.activation(
                        o[:].rearrange("p a b -> p (a b)"),
                        r[0:2, :, :].rearrange("p a b -> p (a b)"),
                        SIGMOID, bias=b2ap, scale=SC2,
                    )
                    nc.sync.dma_start(
                        out_dram[0:2, n0 : n0 + 2 * NCH],
                        o[:].rearrange("p a b -> p (a b)"),
                    )

            # --- software-pipelined main loop over double-chunks: dc+1's L0
            # runs between dc's L1 and L2, giving every relu a full PE-block
            # of slack before its consumer ---
            h1s = [l0(0, 0), l0(1, 0)]
            for dc in range(NDC):
                h2s = [l1(0, h1s[0]), l1(1, h1s[1])]
                if dc + 1 < NDC:
                    h1s = [l0(0, dc + 1), l0(1, dc + 1)]
                l2(dc, h2s)

    nc.compile()
    return nc


_NC_CACHE = None


def _get_nc():
    global _NC_CACHE
    if _NC_CACHE is None:
        _NC_CACHE = _build()
    return _NC_CACHE


def _q8(x, scale):
    return (np.asarray(x, dtype=np.float32) * scale).astype(F8NP)


def _make_in_maps(e_embedding, W0, b0, W1, b1, W2, b2):
    e = np.asarray(e_embedding, dtype=np.float32)
    W0 = np.asarray(W0, dtype=np.float32)
    b0 = np.asarray(b0, dtype=np.float32)
    W1 = np.asarray(W1, dtype=np.float32)
    b1 = np.asarray(b1, dtype=np.float32)
    W2 = np.asarray(W2, dtype=np.float32)
    b2 = np.asarray(b2, dtype=np.float32)

    # eT [E, N] -> [ki, kp, two, n] fp8 (replicated to all cores)
    eTq = np.ascontiguousarray(
        _q8(e.T, SE).reshape(KP, 2, 128, N).transpose(2, 0, 1, 3).reshape(128, -1)
    )

    def wpack(W, s):  # [pp, 512, 512] -> [ki, (p j q two m)] fp8
        return np.ascontiguousarray(
            _q8(W, s)
            .reshape(P_PER, KP, 2, 128, JC, 128)
            .transpose(3, 0, 4, 1, 2, 5)
            .reshape(128, -1)
        )

    in_maps = []
    for cid in range(N_CORES):
        sl = slice(P_PER * cid, P_PER * (cid + 1))
        w0t = wpack(W0[sl], SW0)
        w1t = wpack(W1[sl], SW1)
        w2q = (
            _q8(W2[sl, :, 0], SW2)
            .reshape(P_PER, KP, 2, 128)
            .transpose(3, 0, 1, 2)
        )  # [128, p, q, two]
        w2t = np.zeros((128, P_PER, KP, 2, 64), dtype=F8NP)
        for p in range(P_PER):
            w2t[:, p, :, :, p] = w2q[:, p]
        w2t = np.ascontiguousarray(w2t.reshape(128, -1))
        bbt = np.zeros((128, 2 * P_PER * JC + 1), dtype=np.float32)
        bbt[:, 0:8] = (
            (b0[sl] * SH1).reshape(P_PER, JC, 128).transpose(2, 0, 1).reshape(128, -1)
        )
        bbt[:, 8:16] = (
            (b1[sl] * SH2).reshape(P_PER, JC, 128).transpose(2, 0, 1).reshape(128, -1)
        )
        bbt[0:2, 16] = b2[sl, 0]
        in_maps.append(
            {"eT": eTq, "w0": w0t, "w1": w1t, "w2": w2t,
             "bb": np.ascontiguousarray(bbt)}
        )
    return in_maps


def kernel_with_results(trace=False, **inputs):
    nc = _get_nc()
    in_maps = _make_in_maps(**inputs)
    try:
        res = run_bass_kernel_spmd(
            nc, in_maps, core_ids=list(range(N_CORES)), trace=trace
        )
    except Exception:
        # the first PJRT compile in a fresh container can fail transiently;
        # one retry reuses the primed NEFF cache
        res = run_bass_kernel_spmd(
            nc, in_maps, core_ids=list(range(N_CORES)), trace=trace
        )
    full = np.concatenate([r["out"] for r in res.results], axis=0)  # [16, N]
    out = np.ascontiguousarray(full.T).astype(np.float32)  # [N, 16]
    return out, res


def kernel(**inputs):
    out, _ = kernel_with_results(trace=False, **inputs)
    return out

